# revision 16
# baseline (speedup 1.0000x reference)
"""PatchMatch-style MatchingPropagator on 8 Trainium2 NeuronCores.

Full inputs in, full outputs out. Sharding: 8 independent units =
(direction in {forward, backward}) x (batch 0..3), one NeuronCore each.

Key layout decisions:
- The host re-packs each unit's correlation volume into "quad" records
  Q[n, y0, x0, 0:4] = corr[n, y0:y0+2, x0:x0+2] for anchors in [0,62]^2,
  so every bilinear sample is ONE contiguous 16-byte indirect-DMA fetch.
  Clamping floors to <=62 is numerically identical to the reference's
  corner clamping.
- CT holds three [x|y|s] 96-col blocks (BEST, H-cand, V-cand); accepts
  are one 96-wide broadcast is_gt + one 96-wide copy_predicated.
- Per-corner bilinear weights PW = [u*t, w*t, u*wy, w*wy] are built as a
  single broadcast outer-product multiply from [u w] / [t wy] pair
  tiles (pairs interleaved on the Activation engine, off the critical
  path), so a score eval on the critical path is one contiguous
  multiply + one [e,4] tensor_reduce with the reference's sequential
  sum order s = ((t1+t2)+t3)+t4.
- The random search is fully speculative: candidate coords + indices
  run on the GpSimd engine, the gather is prefetched, and scores for
  all three possible propagate outcomes (S3) are computed the moment
  the gather lands; after the accepts, two 96-wide predicated copies
  select the realized variant and one is_gt + copy_predicated applies
  the update.
- The vertical (row) neighbor roll is a partition shift: an exact
  0/1-permutation f32 matmul on the otherwise-idle PE engine; the
  floor/index chain reads the PSUM result directly so the gather can
  issue before the SBUF candidate field is even materialized.

Pixel layout on chip: pixel (i, j) -> partition 64*(j//32) + i, free j%32.
"""

import numpy as np

B, H, W = 4, 64, 64
R = 3.0
EPS = np.float32(0.01)
N_CORES = 8
PIX = H * W              # 4096 pixels per unit
AN = W - 1               # 63 anchors per axis in the quad layout
QROW = AN * 4            # 252 floats per anchor row
QMAP = AN * AN * 4       # 15876 floats per pixel quad map

_CACHE = {}


# ----------------------------------------------------------------------------
# Device program (SPMD: identical on all 8 cores; data differs per core)
# ----------------------------------------------------------------------------

def _build_program():
    import concourse.bass as bass
    import concourse.mybir as mybir
    import concourse.tile as tile
    from concourse import bacc
    from concourse.bass import MemorySpace

    F32 = mybir.dt.float32
    I32 = mybir.dt.int32
    OP = mybir.AluOpType
    AF = mybir.ActivationFunctionType

    nc = bacc.Bacc(
        "TRN2",
        target_bir_lowering=False,
        debug=False,
        enable_asserts=False,
        num_devices=N_CORES,
    )

    corr = nc.dram_tensor("corr", [PIX * QMAP], F32, kind="ExternalInput")
    # state cols (32 each): [x, y, hx1, hy1, vx1, vy1, base, nx1, ny1,
    #                        nx2, ny2, nx3, ny3] + two 128-col permutation
    # matrices (row-roll +1 / -1) for the PE-based vertical roll
    state_in = nc.dram_tensor("state", [128, 13 * 32 + 256], F32,
                              kind="ExternalInput")
    out_xy = nc.dram_tensor("out_xy", [128, 288], F32,
                            kind="ExternalOutput")

    corr_flat = corr.ap().rearrange("(n one) -> n one", one=1)

    with tile.TileContext(nc) as tc:
        with tc.tile_pool(name="main", bufs=1) as pool, \
             tc.tile_pool(name="psum", bufs=2,
                          space=MemorySpace.PSUM) as ppool:
            ST = pool.tile([128, 13 * 32 + 256], F32, name="ST")
            nc.sync.dma_start(ST[:], state_in.ap())
            PU = ST[:, 416:544]   # row-roll +1 permutation (lhsT)
            PD = ST[:, 544:672]   # row-roll -1 permutation (lhsT)

            def noise_view(k):
                o = 224 + 64 * k
                return ST[:, o:o + 64]  # [nx|ny]

            # CT: [BEST | H | V], each [x|y|s] 96 cols
            CT = pool.tile([128, 288], F32, name="CT")
            # candidate coords [xh yh | xv yv]
            CC = pool.tile([128, 128], F32, name="CC")
            G = pool.tile([128, 768], F32, name="G")     # eval 0:384, spec 384:768
            WT = pool.tile([128, 192], F32, name="WT")   # eval [w(<=96) | wy@96]
            WT3 = pool.tile([128, 192], F32, name="WT3")  # spec [w96 | wy96]
            WU = pool.tile([128, 192], F32, name="WU")   # eval [u w] pairs
            WY = pool.tile([128, 192], F32, name="WY")   # eval [t wy] pairs
            WU3 = pool.tile([128, 192], F32, name="WU3")
            WY3 = pool.tile([128, 192], F32, name="WY3")
            PW = pool.tile([128, 384], F32, name="PW")   # eval corner weights
            PW3 = pool.tile([128, 384], F32, name="PW3")
            B2 = pool.tile([128, 384], F32, name="B2")   # eval products
            B3 = pool.tile([128, 384], F32, name="B3")   # spec products
            XI = pool.tile([128, 192], I32, name="XI")   # eval floored coords
            XIS = pool.tile([128, 192], I32, name="XIS")  # spec floored coords
            IF = pool.tile([128, 96], I32, name="IF")
            I = pool.tile([128, 96], I32, name="I")
            ISF = pool.tile([128, 96], I32, name="ISF")
            IS = pool.tile([128, 96], I32, name="IS")
            UPD = pool.tile([128, 192], I32, name="UPD")  # two 96-wide masks
            RC = pool.tile([128, 288], F32, name="RC")   # spec [x|y|s] x 3
            BASEI = pool.tile([128, 32], I32, name="BASEI")
            WI = pool.tile([128, 32], I32, name="WI")    # warm gather idx
            WG = pool.tile([128, 128], F32, name="WG")   # warm gather dest

            v = nc.vector
            a = nc.scalar
            g = nc.gpsimd

            # warm gather: triggers the gpsimd indirect-DMA library load
            # early; touches only dedicated tiles so nothing later stalls
            # on its completion.
            v.memset(WI[:], 0)
            g.indirect_dma_start(
                out=WG[:],
                out_offset=None,
                in_=corr_flat,
                in_offset=bass.IndirectOffsetOnAxis(ap=WI[:], axis=0),
            )
            v.tensor_copy(BASEI[:], ST[:, 192:224])

            def b3(ap):  # [128,32] -> broadcast [128,3,32]
                return ap.rearrange("p (one f) -> p one f", one=1).to_broadcast(
                    [128, 3, 32])

            def ct_blk(i, n=1):
                """[128, n, 96] view of CT starting at block i."""
                return CT[:].rearrange("p (b f) -> p b f", b=3)[:, i:i + n]

            def rc_blk(i, n=1):
                return RC[:].rearrange("p (b f) -> p b f", b=3)[:, i:i + n]

            def eval_idx(ne):
                """quad indices from the floored coords in XI[0:64*ne]."""
                m = 32 * ne
                x2 = XI[:, 0:2 * m].rearrange("p (c s q) -> p c s q",
                                              c=ne, s=2)
                if3 = IF[:, 0:m].rearrange("p (e q) -> p e q", e=ne)
                i3 = I[:, 0:m].rearrange("p (e q) -> p e q", e=ne)
                baseb = (BASEI.rearrange("p (one f) -> p one f", one=1)
                         .to_broadcast([128, ne, 32]))
                v.scalar_tensor_tensor(if3, x2[:, :, 1], QROW, baseb,
                                       OP.mult, OP.add)
                v.scalar_tensor_tensor(i3, x2[:, :, 0], 4, if3,
                                       OP.mult, OP.add)

            def eval_gather(ne):
                g.indirect_dma_start(
                    out=G[:, 0:128 * ne],
                    out_offset=None,
                    in_=corr_flat,
                    in_offset=bass.IndirectOffsetOnAxis(
                        ap=I[:, 0:32 * ne], axis=0),
                )

            def pairs(wcol, wycol, wut, wyt, m):
                """[u w] / [t wy] pair interleaves on the Act engine."""
                wuv = wut[:, 0:2 * m].rearrange("p (e d) -> p e d", d=2)
                wyv = wyt[:, 0:2 * m].rearrange("p (e d) -> p e d", d=2)
                wc = wcol.rearrange("p (e one) -> p e one", one=1)
                wyc = wycol.rearrange("p (e one) -> p e one", one=1)
                a.copy(wuv[:, :, 1:2], wc)
                a.activation(wuv[:, :, 0:1], wc, AF.Identity, bias=1.0,
                             scale=-1.0)
                a.copy(wyv[:, :, 1:2], wyc)
                a.activation(wyv[:, :, 0:1], wyc, AF.Identity, bias=1.0,
                             scale=-1.0)

            def pw_mult(eng, wut, wyt, pwt, m):
                """corner weights [u*t, w*t, u*wy, w*wy] per pixel as one
                broadcast outer-product multiply."""
                wu_b = (wut[:, 0:2 * m]
                        .rearrange("p (e one d) -> p e one d", one=1, d=2)
                        .to_broadcast([128, m, 2, 2]))
                wy_b = (wyt[:, 0:2 * m]
                        .rearrange("p (e s one) -> p e s one", one=1, s=2)
                        .to_broadcast([128, m, 2, 2]))
                out = pwt[:, 0:4 * m].rearrange("p (e s d) -> p e s d",
                                                s=2, d=2)
                eng.tensor_tensor(out, wu_b, wy_b, OP.mult)

            def weights(cv, ne):
                """w/wy fractions (DVE) + pairs (Act) + PW (DVE), hidden
                under the eval gather's DMA flight."""
                m = 32 * ne
                c2 = cv.rearrange("p (c s q) -> p c s q", c=ne, s=2)
                x2 = (XI[:, 0:2 * m]
                      .rearrange("p (c s q) -> p c s q", c=ne, s=2))
                w = WT[:, 0:m].rearrange("p (e q) -> p e q", e=ne)
                wy = WT[:, 96:96 + m].rearrange("p (e q) -> p e q", e=ne)
                v.tensor_tensor(w, c2[:, :, 0], x2[:, :, 0], OP.subtract)
                v.tensor_tensor(wy, c2[:, :, 1], x2[:, :, 1], OP.subtract)
                pairs(WT[:, 0:m], WT[:, 96:96 + m], WU, WY, m)
                pw_mult(v, WU, WY, PW, m)

            def eval_score(ne):
                """score for `ne` eval slots -> CT s-cols of blocks H.. ."""
                n = 128 * ne
                v.tensor_tensor(B2[:, 0:n], G[:, 0:n], PW[:, 0:n], OP.mult)
                b4 = B2[:, 0:n].rearrange("p (e k) -> p e k", k=4)
                sc = ct_blk(3 - ne, ne)[:, :, 64:96]
                v.tensor_reduce(sc, b4, mybir.AxisListType.X, OP.add)

            def accept(blk):
                """BEST = block blk where its score is higher; one 96-wide
                mask + one 96-wide predicated copy. Mask lands in UPD slot
                blk-1 for the random-search variant selection."""
                mo = 96 * (blk - 1)
                m3 = UPD[:, mo:mo + 96].rearrange("p (c f) -> p c f", c=3)
                v.tensor_tensor(m3, b3(ct_blk(blk)[:, 0, 64:96]),
                                b3(CT[:, 64:96]), OP.is_gt)
                v.copy_predicated(CT[:, 0:96], UPD[:, mo:mo + 96],
                                  ct_blk(blk)[:, 0])

            def spec_coords(k, first=False):
                """Speculative random-search candidate coords + floors for
                all three possible accept outcomes."""
                nzb = (noise_view(k)
                       .rearrange("p (one f) -> p one f", one=1)
                       .to_broadcast([128, 3, 64]))
                rxy = rc_blk(0, 3)[:, :, 0:64]
                if first:
                    # candidates live in ST [x,y,hx,hy,vx,vy]
                    cv3 = ST[:, 0:192].rearrange("p (c f) -> p c f", c=3)
                    v.tensor_tensor(rxy, cv3, nzb, OP.add)
                else:
                    # B from CT, H/V from CC
                    v.tensor_tensor(rc_blk(0)[:, :, 0:64],
                                    ct_blk(0)[:, :, 0:64], nzb[:, 0:1],
                                    OP.add)
                    cc2 = CC[:].rearrange("p (c f) -> p c f", c=2)
                    v.tensor_tensor(rc_blk(1, 2)[:, :, 0:64], cc2,
                                    nzb[:, 0:2], OP.add)
                v.tensor_scalar(rxy, rxy, 0.0, float(W - 1),
                                OP.max, OP.min)
                v.tensor_scalar(XIS[:], rxy, float(AN - 1), None, OP.min)

            def spec_gather():
                """spec quad indices + prefetch gather."""
                x2 = XIS[:].rearrange("p (c s q) -> p c s q", c=3, s=2)
                if3 = ISF[:].rearrange("p (e q) -> p e q", e=3)
                i3 = IS[:].rearrange("p (e q) -> p e q", e=3)
                baseb = (BASEI.rearrange("p (one f) -> p one f", one=1)
                         .to_broadcast([128, 3, 32]))
                v.scalar_tensor_tensor(if3, x2[:, :, 1], QROW, baseb,
                                       OP.mult, OP.add)
                v.scalar_tensor_tensor(i3, x2[:, :, 0], 4, if3,
                                       OP.mult, OP.add)
                g.indirect_dma_start(
                    out=G[:, 384:768],
                    out_offset=None,
                    in_=corr_flat,
                    in_offset=bass.IndirectOffsetOnAxis(ap=IS[:], axis=0),
                )

            def spec_weights():
                """spec weight fractions (DVE) + pairs (Act) + PW3 (DVE),
                hidden under the gathers' flight."""
                rxy = rc_blk(0, 3)[:, :, 0:64]
                x2 = XIS[:].rearrange("p (c s q) -> p c s q", c=3, s=2)
                w3 = WT3[:, 0:96].rearrange("p (c q) -> p c q", c=3)
                wy3 = WT3[:, 96:192].rearrange("p (c q) -> p c q", c=3)
                rc2 = rxy.rearrange("p c (s q) -> p c s q", s=2)
                v.tensor_tensor(w3, rc2[:, :, 0], x2[:, :, 0], OP.subtract)
                v.tensor_tensor(wy3, rc2[:, :, 1], x2[:, :, 1], OP.subtract)
                pairs(WT3[:, 0:96], WT3[:, 96:192], WU3, WY3, 96)
                pw_mult(v, WU3, WY3, PW3, 96)

            def s3_score():
                """Score all three spec variants the moment the spec gather
                lands: products on GpSimd (idle by then), reduce on DVE.
                Emitted early so the DVE's OOO window can fire the reduce
                behind the accepts."""
                g.tensor_tensor(B3[:], G[:, 384:768], PW3[:], OP.mult)
                b4 = B3[:].rearrange("p (e k) -> p e k", k=4)
                s3 = rc_blk(0, 3)[:, :, 64:96]
                v.tensor_reduce(s3, b4, mybir.AxisListType.X, OP.add)

            def rs_finish():
                """Select the realized spec variant by the accept masks,
                apply the random-search update."""
                v.copy_predicated(RC[:, 0:96], UPD[:, 0:96], rc_blk(1)[:, 0])
                v.copy_predicated(RC[:, 0:96], UPD[:, 96:192],
                                  rc_blk(2)[:, 0])
                # accept: new_s > old_s
                m3 = UPD[:, 0:96].rearrange("p (c f) -> p c f", c=3)
                v.tensor_tensor(m3, b3(RC[:, 64:96]), b3(CT[:, 64:96]),
                                OP.is_gt)
                v.copy_predicated(CT[:, 0:96], UPD[:, 0:96], RC[:, 0:96])

            def ct_save():
                """CC -> CT H/V coord cols (Act, under gather flight)."""
                a.copy(CT[:, 96:160], CC[:, 0:64])
                a.copy(CT[:, 192:256], CC[:, 64:128])

            # ---- round 1: initial eval + propagate(1,1); candidates
            # pre-rolled on the host in ST.
            v.tensor_scalar(XI[:, 0:192], ST[:, 0:192], float(AN - 1),
                            None, OP.min)
            eval_idx(3)
            eval_gather(3)
            # CT init: [x|y] of B/H/V from ST's 3 coord pairs
            cxy = (CT[:].rearrange("p (b f) -> p b f", b=3)[:, :, 0:64])
            v.tensor_copy(cxy, ST[:, 0:192].rearrange(
                "p (c f) -> p c f", c=3))
            spec_coords(0, first=True)
            spec_gather()
            weights(ST[:, 0:192], 3)
            spec_weights()
            s3_score()
            eval_score(3)
            accept(1)
            accept(2)
            rs_finish()

            def propagate(dx, dy, spec_k=None):
                src = CT[:, 0:64]
                # ---- v roll: permutation matmul (partition shift) on PE
                ps = ppool.tile([128, 64], F32)
                nc.tensor.matmul(ps[:], PU if dy == 1 else PD, src,
                                 start=True, stop=True)
                # ---- h roll on DVE (free-dim shift + cross-half wrap)
                dh = CC[:, 0:64].rearrange("p (c f) -> p c f", c=2)
                sh = src.rearrange("p (c f) -> p c f", c=2)
                if dx == 1:
                    v.tensor_copy(dh[:, :, 1:32], sh[:, :, 0:31])
                    v.tensor_copy(dh[64:128, :, 0:1], sh[0:64, :, 31:32])
                    v.tensor_copy(dh[0:64, :, 0:1], sh[64:128, :, 31:32])
                    v.tensor_scalar(CC[:, 0:32], CC[:, 0:32], 1.0,
                                    float(W - 1), OP.add, OP.min)
                else:
                    v.tensor_copy(dh[:, :, 0:31], sh[:, :, 1:32])
                    v.tensor_copy(dh[0:64, :, 31:32], sh[64:128, :, 0:1])
                    v.tensor_copy(dh[64:128, :, 31:32], sh[0:64, :, 0:1])
                    v.tensor_scalar(CC[:, 0:32], CC[:, 0:32], -1.0, 0.0,
                                    OP.add, OP.max)
                # ---- floors: H from CC, V straight out of PSUM, so the
                # gather goes out before the V candidate hits SBUF
                v.tensor_scalar(XI[:, 0:64], CC[:, 0:64], float(AN - 1),
                                None, OP.min)
                v.tensor_scalar(XI[:, 64:96], ps[:, 0:32], float(AN - 1),
                                None, OP.min)
                if dy == 1:
                    v.tensor_scalar(XI[:, 96:128], ps[:, 32:64], 1.0,
                                    float(AN - 1), OP.add, OP.min)
                else:
                    v.tensor_scalar(XI[:, 96:128], ps[:, 32:64], -1.0,
                                    0.0, OP.add, OP.max)
                eval_idx(2)
                eval_gather(2)
                # ---- V candidate to SBUF (for spec/weights/saves)
                v.tensor_copy(CC[:, 64:96], ps[:, 0:32])
                if dy == 1:
                    v.tensor_scalar(CC[:, 96:128], ps[:, 32:64], 1.0,
                                    float(H - 1), OP.add, OP.min)
                else:
                    v.tensor_scalar(CC[:, 96:128], ps[:, 32:64], -1.0,
                                    0.0, OP.add, OP.max)
                # hidden under the gather flight
                if spec_k is not None:
                    spec_coords(spec_k)
                    spec_gather()
                weights(CC[:], 2)
                ct_save()
                if spec_k is not None:
                    spec_weights()
                    s3_score()
                eval_score(2)
                if spec_k is None:
                    return  # final accepts run on the host
                accept(1)
                accept(2)
                rs_finish()

            propagate(-1, -1, spec_k=1)
            propagate(-1, 1, spec_k=2)
            propagate(1, -1)

            nc.sync.dma_start(out_xy.ap(), CT[:])

    nc.compile()
    return nc


def _get_program():
    if "nc" not in _CACHE:
        _CACHE["nc"] = _build_program()
    return _CACHE["nc"]


# ----------------------------------------------------------------------------
# Host-side helpers
# ----------------------------------------------------------------------------

def _to_layout(v):
    """[64(i), 64(j)] -> [128, 32]; partition = 64*(j//32)+i, free = j%32."""
    return np.ascontiguousarray(
        v.reshape(64, 2, 32).transpose(1, 0, 2).reshape(128, 32))


def _from_layout(a):
    """[128, 32] -> [64(i), 64(j)]."""
    return a.reshape(2, 64, 32).transpose(1, 0, 2).reshape(64, 64)


def _noise_arrays():
    """Mirror the reference's jax.random usage exactly, in-process, so the
    values match the grader's reference no matter which jax backend/PRNG
    the process defaults to."""
    import jax
    import jax.numpy as jnp

    key = jax.random.key(42)
    kf, kb = jax.random.split(key)
    out = []
    for kdir in (kf, kb):
        ks = jax.random.split(kdir, 3)
        out.append([np.asarray(R * jax.random.normal(k, (B, H, W, 2),
                                                     jnp.float32))
                    for k in ks])
    return out  # [dir][step] -> [B,H,W,2] float32


def _quad_pack(corr_u):
    """[4096, 64, 64] -> flat quad records [4096*63*63*4] f32."""
    sw = np.lib.stride_tricks.sliding_window_view(corr_u, (2, 2),
                                                  axis=(1, 2))
    # sw: [4096, 63, 63, 2, 2]
    return np.ascontiguousarray(sw).reshape(-1)


def _roll_perm_mats():
    """Permutation lhsT matrices for the PE row-roll: out[m] = src[sig(m)]
    with sig(m) = 64*(m//64) + ((m%64 -/+ 1) % 64)."""
    up = np.zeros((128, 128), np.float32)
    dn = np.zeros((128, 128), np.float32)
    for m in range(128):
        blk, i = divmod(m, 64)
        up[64 * blk + (i - 1) % 64, m] = 1.0
        dn[64 * blk + (i + 1) % 64, m] = 1.0
    return up, dn


def _make_state(x_plane, y_plane, noise_steps, b):
    """Build the [128, 13*32+256] per-core state tensor (partition-major)."""
    x = x_plane.astype(np.float32)
    y = y_plane.astype(np.float32)
    one = np.float32(1.0)
    # first propagate is (dx, dy) = (1, 1); host pre-rolls the candidates
    hx = np.clip(np.roll(x, 1, axis=1) + one, np.float32(0.0),
                 np.float32(W - 1))
    hy = np.roll(y, 1, axis=1)
    vx = np.roll(x, 1, axis=0)
    vy = np.clip(np.roll(y, 1, axis=0) + one, np.float32(0.0),
                 np.float32(H - 1))
    base = ((np.arange(64, dtype=np.int64)[:, None] * 64
             + np.arange(64, dtype=np.int64)[None, :]) * QMAP)
    rows = [
        _to_layout(x), _to_layout(y),
        _to_layout(hx), _to_layout(hy),
        _to_layout(vx), _to_layout(vy),
        _to_layout(base.astype(np.float32)),
    ]
    for step in range(3):
        nz = noise_steps[step][b]  # [H,W,2]
        rows.append(_to_layout(np.ascontiguousarray(nz[:, :, 0])))
        rows.append(_to_layout(np.ascontiguousarray(nz[:, :, 1])))
    rows.extend(_roll_perm_mats())
    return np.concatenate(rows, axis=1).astype(np.float32)


def _bilinear_map_np(img, coords):
    """numpy mirror of reference._bilinear_map (fp32, same op order).
    img [B,H,W,C], coords [B,H,W,2] -> [B,H,W,C]"""
    Bn, Hn, Wn, C = img.shape
    out = np.empty_like(img)
    one = np.float32(1.0)
    for b in range(Bn):
        x = coords[b, :, :, 0].reshape(-1)
        y = coords[b, :, :, 1].reshape(-1)
        x0 = np.floor(x)
        y0 = np.floor(y)
        wx = (x - x0)[:, None]
        wy = (y - y0)[:, None]
        x0i = np.clip(x0.astype(np.int32), 0, Wn - 1)
        x1i = np.clip(x0i + 1, 0, Wn - 1)
        y0i = np.clip(y0.astype(np.int32), 0, Hn - 1)
        y1i = np.clip(y0i + 1, 0, Hn - 1)
        im = img[b]
        v00 = im[y0i, x0i]
        v01 = im[y0i, x1i]
        v10 = im[y1i, x0i]
        v11 = im[y1i, x1i]
        o = (v00 * (one - wx) * (one - wy) + v01 * wx * (one - wy)
             + v10 * (one - wx) * wy + v11 * wx * wy)
        out[b] = o.reshape(Hn, Wn, C)
    return out


def _run_device(in_maps, trace=False):
    from concourse import bass_utils

    nc = _get_program()
    res = bass_utils.run_bass_kernel_spmd(
        nc, in_maps, core_ids=list(range(N_CORES)), trace=trace)
    return res


def kernel(matching_f, matching_b, corr_map, _trace=False, _results_hook=None):
    matching_f = np.asarray(matching_f)
    matching_b = np.asarray(matching_b)
    corr_map = np.asarray(corr_map)

    noise = _noise_arrays()  # [dir][step][B,H,W,2]

    in_maps = []
    for b in range(B):  # forward units, cores 0..3
        corr_u = np.ascontiguousarray(corr_map[b]).reshape(PIX, H, W)
        in_maps.append({
            "corr": _quad_pack(corr_u),
            "state": _make_state(matching_f[b, 0], matching_f[b, 1],
                                 noise[0], b),
        })
    for b in range(B):  # backward units, cores 4..7
        corr_t = np.ascontiguousarray(
            corr_map[b].transpose(2, 3, 0, 1)).reshape(PIX, H, W)
        in_maps.append({
            "corr": _quad_pack(corr_t),
            "state": _make_state(matching_b[b, 0], matching_b[b, 1],
                                 noise[1], b),
        })

    res = _run_device(in_maps, trace=_trace)
    if _results_hook is not None:
        _results_hook(res)

    def _final_accepts(of):
        """host mirror of the last propagate's two sequential accepts."""
        xb, yb, sb = of[:, 0:32], of[:, 32:64], of[:, 64:96]
        xh, yh, sh = of[:, 96:128], of[:, 128:160], of[:, 160:192]
        xv, yv, sv = of[:, 192:224], of[:, 224:256], of[:, 256:288]
        u1 = sh > sb
        x1 = np.where(u1, xh, xb)
        y1 = np.where(u1, yh, yb)
        s1 = np.where(u1, sh, sb)
        u2 = sv > s1
        return np.where(u2, xv, x1), np.where(u2, yv, y1)

    res_f = np.empty((B, H, W, 2), np.float32)
    res_b = np.empty((B, H, W, 2), np.float32)
    for b in range(B):
        xf, yf = _final_accepts(res.results[b]["out_xy"])
        xb_, yb_ = _final_accepts(res.results[4 + b]["out_xy"])
        res_f[b, :, :, 0] = _from_layout(xf)
        res_f[b, :, :, 1] = _from_layout(yf)
        res_b[b, :, :, 0] = _from_layout(xb_)
        res_b[b, :, :, 1] = _from_layout(yb_)

    # forward-backward consistency (host; mirrors reference in fp32)
    counter = _bilinear_map_np(res_b, res_f)
    diff = np.max(np.abs(res_f - counter), axis=-1)
    invalid = (diff > EPS)[..., None]
    mf_t = matching_f.transpose(0, 2, 3, 1)  # [B,H,W,2]
    out = np.where(invalid, mf_t, res_f)
    return np.ascontiguousarray(out.transpose(0, 3, 1, 2)).astype(np.float32)


# revision 18
# speedup vs baseline: 1.0631x; 1.0631x over previous
"""PatchMatch-style MatchingPropagator on 8 Trainium2 NeuronCores.

Full inputs in, full outputs out. Sharding: 8 independent units =
(direction in {forward, backward}) x (batch 0..3), one NeuronCore each.

Key layout decisions:
- The host re-packs each unit's correlation volume into "quad" records
  Q[n, y0, x0, 0:4] = corr[n, y0:y0+2, x0:x0+2] for anchors in [0,62]^2,
  so every bilinear sample is ONE contiguous 16-byte indirect-DMA fetch.
  Clamping floors to <=62 is numerically identical to the reference's
  corner clamping.
- CT holds three [x|y|s] 96-col blocks (BEST, H-cand, V-cand); accepts
  are one 96-wide broadcast is_gt + one 96-wide copy_predicated.
- Per-corner bilinear weights PW = [u*t, w*t, u*wy, w*wy] are built as a
  single broadcast outer-product multiply from [u w] / [t wy] pair
  tiles (pairs interleaved on the Activation engine, off the critical
  path), so a score eval on the critical path is one contiguous
  multiply + one [e,4] tensor_reduce with the reference's sequential
  sum order s = ((t1+t2)+t3)+t4.
- The random search is fully speculative: candidate coords + indices
  run on the GpSimd engine, the gather is prefetched, and scores for
  all three possible propagate outcomes (S3) are computed the moment
  the gather lands; after the accepts, two 96-wide predicated copies
  select the realized variant and one is_gt + copy_predicated applies
  the update.
- The vertical (row) neighbor roll is a partition shift: an exact
  0/1-permutation f32 matmul on the otherwise-idle PE engine; the
  floor/index chain reads the PSUM result directly so the gather can
  issue before the SBUF candidate field is even materialized.

Pixel layout on chip: pixel (i, j) -> partition 64*(j//32) + i, free j%32.
"""

import numpy as np

B, H, W = 4, 64, 64
R = 3.0
EPS = np.float32(0.01)
N_CORES = 8
PIX = H * W              # 4096 pixels per unit
AN = W - 1               # 63 anchors per axis in the quad layout
QROW = AN * 4            # 252 floats per anchor row
QMAP = AN * AN * 4       # 15876 floats per pixel quad map

_CACHE = {}


# ----------------------------------------------------------------------------
# Device program (SPMD: identical on all 8 cores; data differs per core)
# ----------------------------------------------------------------------------

def _build_program():
    import concourse.bass as bass
    import concourse.mybir as mybir
    import concourse.tile as tile
    from concourse import bacc
    from concourse.bass import MemorySpace

    F32 = mybir.dt.float32
    I32 = mybir.dt.int32
    OP = mybir.AluOpType
    AF = mybir.ActivationFunctionType

    nc = bacc.Bacc(
        "TRN2",
        target_bir_lowering=False,
        debug=False,
        enable_asserts=False,
        num_devices=N_CORES,
    )

    corr = nc.dram_tensor("corr", [PIX * QMAP], F32, kind="ExternalInput")
    # state cols (32 each): [x, y, hx1, hy1, vx1, vy1, base, nx1, ny1,
    #                        nx2, ny2, nx3, ny3] + two 128-col permutation
    # matrices (row-roll +1 / -1) for the PE-based vertical roll
    state_in = nc.dram_tensor("state", [128, 13 * 32 + 256], F32,
                              kind="ExternalInput")
    out_xy = nc.dram_tensor("out_xy", [128, 288], F32,
                            kind="ExternalOutput")

    corr_flat = corr.ap().rearrange("(n one) -> n one", one=1)

    with tile.TileContext(nc) as tc:
        with tc.tile_pool(name="main", bufs=1) as pool, \
             tc.tile_pool(name="psum", bufs=2,
                          space=MemorySpace.PSUM) as ppool:
            ST = pool.tile([128, 13 * 32 + 256], F32, name="ST")
            nc.sync.dma_start(ST[:], state_in.ap())
            PU = ST[:, 416:544]   # row-roll +1 permutation (lhsT)
            PD = ST[:, 544:672]   # row-roll -1 permutation (lhsT)

            def noise_view(k):
                o = 224 + 64 * k
                return ST[:, o:o + 64]  # [nx|ny]

            # CT: [BEST | H | V], each [x|y|s] 96 cols
            CT = pool.tile([128, 288], F32, name="CT")
            # candidate coords [xh yh | xv yv]
            CC = pool.tile([128, 128], F32, name="CC")
            G = pool.tile([128, 768], F32, name="G")     # eval 0:384, spec 384:768
            WT = pool.tile([128, 192], F32, name="WT")   # eval [w(<=96) | wy@96]
            WT3 = pool.tile([128, 192], F32, name="WT3")  # spec [w96 | wy96]
            WU = pool.tile([128, 192], F32, name="WU")   # eval [u w] pairs
            WY = pool.tile([128, 192], F32, name="WY")   # eval [t wy] pairs
            WU3 = pool.tile([128, 192], F32, name="WU3")
            WY3 = pool.tile([128, 192], F32, name="WY3")
            PW = pool.tile([128, 384], F32, name="PW")   # eval corner weights
            PW3 = pool.tile([128, 384], F32, name="PW3")
            B2 = pool.tile([128, 384], F32, name="B2")   # eval products
            B3 = pool.tile([128, 384], F32, name="B3")   # spec products
            XI = pool.tile([128, 192], I32, name="XI")   # eval floored coords
            XIS = pool.tile([128, 192], I32, name="XIS")  # spec floored coords
            IF = pool.tile([128, 96], I32, name="IF")
            I = pool.tile([128, 96], I32, name="I")
            ISF = pool.tile([128, 96], I32, name="ISF")
            IS = pool.tile([128, 96], I32, name="IS")
            UPD = pool.tile([128, 192], I32, name="UPD")  # two 96-wide masks
            RC = pool.tile([128, 288], F32, name="RC")   # spec [x|y|s] x 3
            BASEI = pool.tile([128, 32], I32, name="BASEI")
            WI = pool.tile([128, 32], I32, name="WI")    # warm gather idx
            WG = pool.tile([128, 128], F32, name="WG")   # warm gather dest

            v = nc.vector
            a = nc.scalar
            g = nc.gpsimd

            # warm gather: triggers the gpsimd indirect-DMA library load
            # early; touches only dedicated tiles so nothing later stalls
            # on its completion.
            v.memset(WI[:], 0)
            g.indirect_dma_start(
                out=WG[:],
                out_offset=None,
                in_=corr_flat,
                in_offset=bass.IndirectOffsetOnAxis(ap=WI[:], axis=0),
            )
            v.tensor_copy(BASEI[:], ST[:, 192:224])

            def b3(ap):  # [128,32] -> broadcast [128,3,32]
                return ap.rearrange("p (one f) -> p one f", one=1).to_broadcast(
                    [128, 3, 32])

            def ct_blk(i, n=1):
                """[128, n, 96] view of CT starting at block i."""
                return CT[:].rearrange("p (b f) -> p b f", b=3)[:, i:i + n]

            def rc_blk(i, n=1):
                return RC[:].rearrange("p (b f) -> p b f", b=3)[:, i:i + n]

            def eval_idx(ne):
                """quad indices from the floored coords in XI[0:64*ne]."""
                m = 32 * ne
                x2 = XI[:, 0:2 * m].rearrange("p (c s q) -> p c s q",
                                              c=ne, s=2)
                if3 = IF[:, 0:m].rearrange("p (e q) -> p e q", e=ne)
                i3 = I[:, 0:m].rearrange("p (e q) -> p e q", e=ne)
                baseb = (BASEI.rearrange("p (one f) -> p one f", one=1)
                         .to_broadcast([128, ne, 32]))
                v.scalar_tensor_tensor(if3, x2[:, :, 1], QROW, baseb,
                                       OP.mult, OP.add)
                v.scalar_tensor_tensor(i3, x2[:, :, 0], 4, if3,
                                       OP.mult, OP.add)

            def eval_gather(ne):
                g.indirect_dma_start(
                    out=G[:, 0:128 * ne],
                    out_offset=None,
                    in_=corr_flat,
                    in_offset=bass.IndirectOffsetOnAxis(
                        ap=I[:, 0:32 * ne], axis=0),
                )

            def pairs(wcol, wycol, wut, wyt, m):
                """[u w] / [t wy] pair interleaves on the Act engine."""
                wuv = wut[:, 0:2 * m].rearrange("p (e d) -> p e d", d=2)
                wyv = wyt[:, 0:2 * m].rearrange("p (e d) -> p e d", d=2)
                wc = wcol.rearrange("p (e one) -> p e one", one=1)
                wyc = wycol.rearrange("p (e one) -> p e one", one=1)
                a.copy(wuv[:, :, 1:2], wc)
                a.activation(wuv[:, :, 0:1], wc, AF.Identity, bias=1.0,
                             scale=-1.0)
                a.copy(wyv[:, :, 1:2], wyc)
                a.activation(wyv[:, :, 0:1], wyc, AF.Identity, bias=1.0,
                             scale=-1.0)

            def pw_mult(eng, wut, wyt, pwt, m):
                """corner weights [u*t, w*t, u*wy, w*wy] per pixel as one
                broadcast outer-product multiply."""
                wu_b = (wut[:, 0:2 * m]
                        .rearrange("p (e one d) -> p e one d", one=1, d=2)
                        .to_broadcast([128, m, 2, 2]))
                wy_b = (wyt[:, 0:2 * m]
                        .rearrange("p (e s one) -> p e s one", one=1, s=2)
                        .to_broadcast([128, m, 2, 2]))
                out = pwt[:, 0:4 * m].rearrange("p (e s d) -> p e s d",
                                                s=2, d=2)
                eng.tensor_tensor(out, wu_b, wy_b, OP.mult)

            def weights(cv, ne):
                """w/wy fractions (DVE) + pairs (Act) + PW (DVE), hidden
                under the eval gather's DMA flight."""
                m = 32 * ne
                c2 = cv.rearrange("p (c s q) -> p c s q", c=ne, s=2)
                x2 = (XI[:, 0:2 * m]
                      .rearrange("p (c s q) -> p c s q", c=ne, s=2))
                w = WT[:, 0:m].rearrange("p (e q) -> p e q", e=ne)
                wy = WT[:, 96:96 + m].rearrange("p (e q) -> p e q", e=ne)
                v.tensor_tensor(w, c2[:, :, 0], x2[:, :, 0], OP.subtract)
                v.tensor_tensor(wy, c2[:, :, 1], x2[:, :, 1], OP.subtract)
                pairs(WT[:, 0:m], WT[:, 96:96 + m], WU, WY, m)
                pw_mult(v, WU, WY, PW, m)

            def eval_score(ne):
                """score for `ne` eval slots -> CT s-cols of blocks H.. ."""
                n = 128 * ne
                v.tensor_tensor(B2[:, 0:n], G[:, 0:n], PW[:, 0:n], OP.mult)
                b4 = B2[:, 0:n].rearrange("p (e k) -> p e k", k=4)
                sc = ct_blk(3 - ne, ne)[:, :, 64:96]
                v.tensor_reduce(sc, b4, mybir.AxisListType.X, OP.add)

            def accept(blk):
                """BEST = block blk where its score is higher; one 96-wide
                mask + one 96-wide predicated copy. Mask lands in UPD slot
                blk-1 for the random-search variant selection."""
                mo = 96 * (blk - 1)
                m3 = UPD[:, mo:mo + 96].rearrange("p (c f) -> p c f", c=3)
                v.tensor_tensor(m3, b3(ct_blk(blk)[:, 0, 64:96]),
                                b3(CT[:, 64:96]), OP.is_gt)
                v.copy_predicated(CT[:, 0:96], UPD[:, mo:mo + 96],
                                  ct_blk(blk)[:, 0])

            def spec_coords(k, first=False):
                """Speculative random-search candidate coords + floors for
                all three possible accept outcomes."""
                nzb = (noise_view(k)
                       .rearrange("p (one f) -> p one f", one=1)
                       .to_broadcast([128, 3, 64]))
                rxy = rc_blk(0, 3)[:, :, 0:64]
                if first:
                    # candidates live in ST [x,y,hx,hy,vx,vy]
                    cv3 = ST[:, 0:192].rearrange("p (c f) -> p c f", c=3)
                    v.tensor_tensor(rxy, cv3, nzb, OP.add)
                else:
                    # B from CT, H/V from CC
                    v.tensor_tensor(rc_blk(0)[:, :, 0:64],
                                    ct_blk(0)[:, :, 0:64], nzb[:, 0:1],
                                    OP.add)
                    cc2 = CC[:].rearrange("p (c f) -> p c f", c=2)
                    v.tensor_tensor(rc_blk(1, 2)[:, :, 0:64], cc2,
                                    nzb[:, 0:2], OP.add)
                v.tensor_scalar(rxy, rxy, 0.0, float(W - 1),
                                OP.max, OP.min)
                v.tensor_scalar(XIS[:], rxy, float(AN - 1), None, OP.min)

            def spec_gather():
                """spec quad indices + prefetch gather."""
                x2 = XIS[:].rearrange("p (c s q) -> p c s q", c=3, s=2)
                if3 = ISF[:].rearrange("p (e q) -> p e q", e=3)
                i3 = IS[:].rearrange("p (e q) -> p e q", e=3)
                baseb = (BASEI.rearrange("p (one f) -> p one f", one=1)
                         .to_broadcast([128, 3, 32]))
                v.scalar_tensor_tensor(if3, x2[:, :, 1], QROW, baseb,
                                       OP.mult, OP.add)
                v.scalar_tensor_tensor(i3, x2[:, :, 0], 4, if3,
                                       OP.mult, OP.add)
                g.indirect_dma_start(
                    out=G[:, 384:768],
                    out_offset=None,
                    in_=corr_flat,
                    in_offset=bass.IndirectOffsetOnAxis(ap=IS[:], axis=0),
                )

            def spec_weights():
                """spec weight fractions (DVE) + pairs (Act) + PW3 (DVE),
                hidden under the gathers' flight."""
                rxy = rc_blk(0, 3)[:, :, 0:64]
                x2 = XIS[:].rearrange("p (c s q) -> p c s q", c=3, s=2)
                w3 = WT3[:, 0:96].rearrange("p (c q) -> p c q", c=3)
                wy3 = WT3[:, 96:192].rearrange("p (c q) -> p c q", c=3)
                rc2 = rxy.rearrange("p c (s q) -> p c s q", s=2)
                v.tensor_tensor(w3, rc2[:, :, 0], x2[:, :, 0], OP.subtract)
                v.tensor_tensor(wy3, rc2[:, :, 1], x2[:, :, 1], OP.subtract)
                pairs(WT3[:, 0:96], WT3[:, 96:192], WU3, WY3, 96)
                pw_mult(v, WU3, WY3, PW3, 96)

            def s3_score():
                """Score all three spec variants the moment the spec gather
                lands. Emitted early so the DVE's OOO window can fire it
                behind the accepts."""
                v.tensor_tensor(B3[:], G[:, 384:768], PW3[:], OP.mult)
                b4 = B3[:].rearrange("p (e k) -> p e k", k=4)
                s3 = rc_blk(0, 3)[:, :, 64:96]
                v.tensor_reduce(s3, b4, mybir.AxisListType.X, OP.add)

            def rs_finish():
                """Select the realized spec variant by the accept masks,
                apply the random-search update."""
                v.copy_predicated(RC[:, 0:96], UPD[:, 0:96], rc_blk(1)[:, 0])
                v.copy_predicated(RC[:, 0:96], UPD[:, 96:192],
                                  rc_blk(2)[:, 0])
                # accept: new_s > old_s
                m3 = UPD[:, 0:96].rearrange("p (c f) -> p c f", c=3)
                v.tensor_tensor(m3, b3(RC[:, 64:96]), b3(CT[:, 64:96]),
                                OP.is_gt)
                v.copy_predicated(CT[:, 0:96], UPD[:, 0:96], RC[:, 0:96])

            def ct_save():
                """CC -> CT H/V coord cols (Act, under gather flight)."""
                a.copy(CT[:, 96:160], CC[:, 0:64])
                a.copy(CT[:, 192:256], CC[:, 64:128])

            # ---- round 1: initial eval + propagate(1,1); candidates
            # pre-rolled on the host in ST.
            v.tensor_scalar(XI[:, 0:192], ST[:, 0:192], float(AN - 1),
                            None, OP.min)
            eval_idx(3)
            eval_gather(3)
            # CT init: [x|y] of B/H/V from ST's 3 coord pairs
            cxy = (CT[:].rearrange("p (b f) -> p b f", b=3)[:, :, 0:64])
            v.tensor_copy(cxy, ST[:, 0:192].rearrange(
                "p (c f) -> p c f", c=3))
            spec_coords(0, first=True)
            spec_gather()
            weights(ST[:, 0:192], 3)
            spec_weights()
            s3_score()
            eval_score(3)
            accept(1)
            accept(2)
            rs_finish()

            def propagate(dx, dy, spec_k=None):
                src = CT[:, 0:64]
                # ---- v roll: permutation matmul (partition shift) on PE
                ps = ppool.tile([128, 64], F32)
                nc.tensor.matmul(ps[:], PU if dy == 1 else PD, src,
                                 start=True, stop=True)
                # ---- h roll on DVE (free-dim shift + cross-half wrap)
                dh = CC[:, 0:64].rearrange("p (c f) -> p c f", c=2)
                sh = src.rearrange("p (c f) -> p c f", c=2)
                if dx == 1:
                    v.tensor_copy(dh[:, :, 1:32], sh[:, :, 0:31])
                    v.tensor_copy(dh[64:128, :, 0:1], sh[0:64, :, 31:32])
                    v.tensor_copy(dh[0:64, :, 0:1], sh[64:128, :, 31:32])
                    v.tensor_scalar(CC[:, 0:32], CC[:, 0:32], 1.0,
                                    float(W - 1), OP.add, OP.min)
                else:
                    v.tensor_copy(dh[:, :, 0:31], sh[:, :, 1:32])
                    v.tensor_copy(dh[0:64, :, 31:32], sh[64:128, :, 0:1])
                    v.tensor_copy(dh[64:128, :, 31:32], sh[0:64, :, 0:1])
                    v.tensor_scalar(CC[:, 0:32], CC[:, 0:32], -1.0, 0.0,
                                    OP.add, OP.max)
                # ---- floors: H from CC, V straight out of PSUM, so the
                # gather goes out before the V candidate hits SBUF
                v.tensor_scalar(XI[:, 0:64], CC[:, 0:64], float(AN - 1),
                                None, OP.min)
                v.tensor_scalar(XI[:, 64:96], ps[:, 0:32], float(AN - 1),
                                None, OP.min)
                if dy == 1:
                    v.tensor_scalar(XI[:, 96:128], ps[:, 32:64], 1.0,
                                    float(AN - 1), OP.add, OP.min)
                else:
                    v.tensor_scalar(XI[:, 96:128], ps[:, 32:64], -1.0,
                                    0.0, OP.add, OP.max)
                eval_idx(2)
                eval_gather(2)
                # ---- V candidate to SBUF (for spec/weights/saves)
                v.tensor_copy(CC[:, 64:96], ps[:, 0:32])
                if dy == 1:
                    v.tensor_scalar(CC[:, 96:128], ps[:, 32:64], 1.0,
                                    float(H - 1), OP.add, OP.min)
                else:
                    v.tensor_scalar(CC[:, 96:128], ps[:, 32:64], -1.0,
                                    0.0, OP.add, OP.max)
                # hidden under the gather flight; spec chain first so its
                # gather ships as early as possible
                if spec_k is not None:
                    spec_coords(spec_k)
                    spec_gather()
                    weights(CC[:], 2)
                    ct_save()
                    spec_weights()
                    s3_score()
                else:
                    weights(CC[:], 2)
                    ct_save()
                eval_score(2)
                if spec_k is None:
                    return  # final accepts run on the host
                accept(1)
                accept(2)
                rs_finish()

            propagate(-1, -1, spec_k=1)
            propagate(-1, 1, spec_k=2)
            propagate(1, -1)

            nc.sync.dma_start(out_xy.ap(), CT[:])

    nc.compile()
    return nc


def _get_program():
    if "nc" not in _CACHE:
        _CACHE["nc"] = _build_program()
    return _CACHE["nc"]


# ----------------------------------------------------------------------------
# Host-side helpers
# ----------------------------------------------------------------------------

def _to_layout(v):
    """[64(i), 64(j)] -> [128, 32]; partition = 64*(j//32)+i, free = j%32."""
    return np.ascontiguousarray(
        v.reshape(64, 2, 32).transpose(1, 0, 2).reshape(128, 32))


def _from_layout(a):
    """[128, 32] -> [64(i), 64(j)]."""
    return a.reshape(2, 64, 32).transpose(1, 0, 2).reshape(64, 64)


def _noise_arrays():
    """Mirror the reference's jax.random usage exactly, in-process, so the
    values match the grader's reference no matter which jax backend/PRNG
    the process defaults to."""
    import jax
    import jax.numpy as jnp

    key = jax.random.key(42)
    kf, kb = jax.random.split(key)
    out = []
    for kdir in (kf, kb):
        ks = jax.random.split(kdir, 3)
        out.append([np.asarray(R * jax.random.normal(k, (B, H, W, 2),
                                                     jnp.float32))
                    for k in ks])
    return out  # [dir][step] -> [B,H,W,2] float32


def _quad_pack(corr_u):
    """[4096, 64, 64] -> flat quad records [4096*63*63*4] f32."""
    sw = np.lib.stride_tricks.sliding_window_view(corr_u, (2, 2),
                                                  axis=(1, 2))
    # sw: [4096, 63, 63, 2, 2]
    return np.ascontiguousarray(sw).reshape(-1)


def _roll_perm_mats():
    """Permutation lhsT matrices for the PE row-roll: out[m] = src[sig(m)]
    with sig(m) = 64*(m//64) + ((m%64 -/+ 1) % 64)."""
    up = np.zeros((128, 128), np.float32)
    dn = np.zeros((128, 128), np.float32)
    for m in range(128):
        blk, i = divmod(m, 64)
        up[64 * blk + (i - 1) % 64, m] = 1.0
        dn[64 * blk + (i + 1) % 64, m] = 1.0
    return up, dn


def _make_state(x_plane, y_plane, noise_steps, b):
    """Build the [128, 13*32+256] per-core state tensor (partition-major)."""
    x = x_plane.astype(np.float32)
    y = y_plane.astype(np.float32)
    one = np.float32(1.0)
    # first propagate is (dx, dy) = (1, 1); host pre-rolls the candidates
    hx = np.clip(np.roll(x, 1, axis=1) + one, np.float32(0.0),
                 np.float32(W - 1))
    hy = np.roll(y, 1, axis=1)
    vx = np.roll(x, 1, axis=0)
    vy = np.clip(np.roll(y, 1, axis=0) + one, np.float32(0.0),
                 np.float32(H - 1))
    base = ((np.arange(64, dtype=np.int64)[:, None] * 64
             + np.arange(64, dtype=np.int64)[None, :]) * QMAP)
    rows = [
        _to_layout(x), _to_layout(y),
        _to_layout(hx), _to_layout(hy),
        _to_layout(vx), _to_layout(vy),
        _to_layout(base.astype(np.float32)),
    ]
    for step in range(3):
        nz = noise_steps[step][b]  # [H,W,2]
        rows.append(_to_layout(np.ascontiguousarray(nz[:, :, 0])))
        rows.append(_to_layout(np.ascontiguousarray(nz[:, :, 1])))
    rows.extend(_roll_perm_mats())
    return np.concatenate(rows, axis=1).astype(np.float32)


def _bilinear_map_np(img, coords):
    """numpy mirror of reference._bilinear_map (fp32, same op order).
    img [B,H,W,C], coords [B,H,W,2] -> [B,H,W,C]"""
    Bn, Hn, Wn, C = img.shape
    out = np.empty_like(img)
    one = np.float32(1.0)
    for b in range(Bn):
        x = coords[b, :, :, 0].reshape(-1)
        y = coords[b, :, :, 1].reshape(-1)
        x0 = np.floor(x)
        y0 = np.floor(y)
        wx = (x - x0)[:, None]
        wy = (y - y0)[:, None]
        x0i = np.clip(x0.astype(np.int32), 0, Wn - 1)
        x1i = np.clip(x0i + 1, 0, Wn - 1)
        y0i = np.clip(y0.astype(np.int32), 0, Hn - 1)
        y1i = np.clip(y0i + 1, 0, Hn - 1)
        im = img[b]
        v00 = im[y0i, x0i]
        v01 = im[y0i, x1i]
        v10 = im[y1i, x0i]
        v11 = im[y1i, x1i]
        o = (v00 * (one - wx) * (one - wy) + v01 * wx * (one - wy)
             + v10 * (one - wx) * wy + v11 * wx * wy)
        out[b] = o.reshape(Hn, Wn, C)
    return out


def _run_device(in_maps, trace=False):
    from concourse import bass_utils

    nc = _get_program()
    res = bass_utils.run_bass_kernel_spmd(
        nc, in_maps, core_ids=list(range(N_CORES)), trace=trace)
    return res


def kernel(matching_f, matching_b, corr_map, _trace=False, _results_hook=None):
    matching_f = np.asarray(matching_f)
    matching_b = np.asarray(matching_b)
    corr_map = np.asarray(corr_map)

    noise = _noise_arrays()  # [dir][step][B,H,W,2]

    in_maps = []
    for b in range(B):  # forward units, cores 0..3
        corr_u = np.ascontiguousarray(corr_map[b]).reshape(PIX, H, W)
        in_maps.append({
            "corr": _quad_pack(corr_u),
            "state": _make_state(matching_f[b, 0], matching_f[b, 1],
                                 noise[0], b),
        })
    for b in range(B):  # backward units, cores 4..7
        corr_t = np.ascontiguousarray(
            corr_map[b].transpose(2, 3, 0, 1)).reshape(PIX, H, W)
        in_maps.append({
            "corr": _quad_pack(corr_t),
            "state": _make_state(matching_b[b, 0], matching_b[b, 1],
                                 noise[1], b),
        })

    res = _run_device(in_maps, trace=_trace)
    if _results_hook is not None:
        _results_hook(res)

    def _final_accepts(of):
        """host mirror of the last propagate's two sequential accepts."""
        xb, yb, sb = of[:, 0:32], of[:, 32:64], of[:, 64:96]
        xh, yh, sh = of[:, 96:128], of[:, 128:160], of[:, 160:192]
        xv, yv, sv = of[:, 192:224], of[:, 224:256], of[:, 256:288]
        u1 = sh > sb
        x1 = np.where(u1, xh, xb)
        y1 = np.where(u1, yh, yb)
        s1 = np.where(u1, sh, sb)
        u2 = sv > s1
        return np.where(u2, xv, x1), np.where(u2, yv, y1)

    res_f = np.empty((B, H, W, 2), np.float32)
    res_b = np.empty((B, H, W, 2), np.float32)
    for b in range(B):
        xf, yf = _final_accepts(res.results[b]["out_xy"])
        xb_, yb_ = _final_accepts(res.results[4 + b]["out_xy"])
        res_f[b, :, :, 0] = _from_layout(xf)
        res_f[b, :, :, 1] = _from_layout(yf)
        res_b[b, :, :, 0] = _from_layout(xb_)
        res_b[b, :, :, 1] = _from_layout(yb_)

    # forward-backward consistency (host; mirrors reference in fp32)
    counter = _bilinear_map_np(res_b, res_f)
    diff = np.max(np.abs(res_f - counter), axis=-1)
    invalid = (diff > EPS)[..., None]
    mf_t = matching_f.transpose(0, 2, 3, 1)  # [B,H,W,2]
    out = np.where(invalid, mf_t, res_f)
    return np.ascontiguousarray(out.transpose(0, 3, 1, 2)).astype(np.float32)
